# revision 4
# baseline (speedup 1.0000x reference)
"""Trainium2 Bass kernel for nn_DiagnosticRNN (embedding GEMM + LSTM + FC).

Data parallel over batch across 8 NeuronCores. Inside each core:
  - messages [2048, 64, 25] are padded host-side to v=32 (channel 25 = const 1.0
    which carries the gate biases through the x-projection matmul).
  - The embedding matmul is folded into the input projection:
        Wx = embedding @ W_ih.T   (so xproj = messages @ Wx, contraction over v)
  - Layout: batch 2048 = 2 streams x 1024; each stream's 1024 batch is stacked
    as [128 partitions = (batch-half0 h-dim | batch-half1 h-dim), 512 columns].
    Gates live in per-function PSUM tiles ([i|f] pair, g, o) so every ACT op
    runs on full 128 partitions.
  - x-projection: one K=64 block-diagonal matmul per gate, reading per-step
    X tiles [64 = (32v half0 | 32v half1), 512] assembled by PE transpose +
    SBUF->SBUF DMA rearrange; recurrence: K=128 block-diagonal W_hh matmuls.
  - All matmul operands are float32r (~1.4e-4 rel err, full PE rate at N=512).
"""

import sys

sys.path.insert(0, "/opt/trn_rl_repo")

import numpy as np

B, S, V, E, H, C = 16384, 64, 25, 64, 64, 3
N_CORES = 8
BC = B // N_CORES  # 2048 batch per core
VP = 32  # padded v: 25 data + 1 const-one (bias) + 6 zeros
N_SG = 2  # independent streams per core
SGB = BC // N_SG  # 1024 batch per stream
NCOL = SGB // 2  # 512 columns (free dim) per stream tile
N_WIN = S // 4  # 16 windows of 4 steps (128 f-columns each)

_CACHE = {}


def _build_program():
    import concourse.mybir as mybir
    import concourse.tile as tile
    from concourse import bacc

    F32 = mybir.dt.float32
    F32R = mybir.dt.float32r
    AF = mybir.ActivationFunctionType

    nc = bacc.Bacc("TRN2", target_bir_lowering=False, debug=False,
                   num_devices=N_CORES)

    msgs_d = nc.declare_dram_parameter("msgs", [BC, S * VP], F32, isOutput=False)
    wx_d = nc.declare_dram_parameter("wx", [64, 4 * 128], F32R, isOutput=False)
    whh_d = nc.declare_dram_parameter("whh", [128, 4 * 128], F32R, isOutput=False)
    wfc_d = nc.declare_dram_parameter("wfc", [128, 8], F32R, isOutput=False)
    fcb_d = nc.declare_dram_parameter("fcb", [8, 1], F32, isOutput=False)
    ident_d = nc.declare_dram_parameter("ident", [128, 128], F32, isOutput=False)
    out_d = nc.declare_dram_parameter("out", [N_SG, 8, NCOL], F32, isOutput=True)

    GATES = ("i", "f", "g", "o")

    with tile.TileContext(nc) as tc:
        with (
            tc.tile_pool(name="const", bufs=1) as cpool,
            tc.tile_pool(name="sb", bufs=2) as sb,
            tc.tile_pool(name="state", bufs=1) as state,
            tc.tile_pool(name="ps", bufs=1, space="PSUM") as ps,
        ):
            wx = cpool.tile([64, 4 * 128], F32R)
            whh = cpool.tile([128, 4 * 128], F32R)
            wfc = cpool.tile([128, 8], F32R)
            fcb = cpool.tile([8, 1], F32)
            ident = cpool.tile([128, 128], F32)
            nc.sync.dma_start(out=wx[:], in_=wx_d[:])
            nc.sync.dma_start(out=whh[:], in_=whh_d[:])
            nc.sync.dma_start(out=wfc[:], in_=wfc_d[:])
            nc.sync.dma_start(out=fcb[:], in_=fcb_d[:])
            nc.sync.dma_start(out=ident[:], in_=ident_d[:])

            # Persistent state per stream.
            Cst = [state.tile([128, NCOL], F32, tag=f"C{sg}", name=f"Cst{sg}")
                   for sg in range(N_SG)]
            Hst = [state.tile([128, NCOL], F32R, tag=f"H{sg}", name=f"Hst{sg}")
                   for sg in range(N_SG)]
            for sg in range(N_SG):
                nc.vector.memset(Cst[sg][:], 0.0)

            msgs2d = msgs_d  # [BC, S*VP]; f index = s*VP + v

            xtiles = [[None] * N_WIN for _ in range(N_SG)]  # per-step X tiles

            def prep_window(sg, w):
                """Load + transpose one 4-step window of messages for stream sg.

                X_raw: [128 part = (j*32+v), 1024 col = half0|half1], then DMA-
                rearranged into per-step tiles [64 = (32v h0 | 32v h1), 512].
                """
                xraw = sb.tile([128, 2 * NCOL], F32R, tag=f"x{sg}")
                for half in range(2):
                    ptr = ps.tile([128, 512], F32, tag=f"g{sg}")
                    for k in range(4):
                        mt = sb.tile([128, 128], F32, tag=f"m{sg}")
                        row0 = sg * SGB + half * NCOL + 128 * k
                        nc.sync.dma_start(
                            out=mt[:],
                            in_=msgs2d[row0:row0 + 128, 128 * w:128 * (w + 1)],
                        )
                        nc.tensor.transpose(ptr[:, 128 * k:128 * (k + 1)], mt[:],
                                            ident[:])
                    if half == 0:
                        nc.vector.tensor_copy(xraw[:, 0:NCOL], ptr[:])
                    else:
                        nc.scalar.copy(xraw[:, NCOL:2 * NCOL], ptr[:])
                steps = []
                for j in range(4):
                    xs = sb.tile([64, NCOL], F32R, tag=f"xs{sg}", bufs=8,
                                 name=f"xs{sg}_{w}_{j}")
                    for half in range(2):
                        nc.sync.dma_start(
                            out=xs[32 * half:32 * half + 32, :],
                            in_=xraw[32 * j:32 * j + 32,
                                     NCOL * half:NCOL * half + NCOL],
                        )
                    steps.append(xs)
                xtiles[sg][w] = steps

            def emit_step(sg, s):
                w, j = divmod(s, 4)
                xs = xtiles[sg][w][j]
                pif = ps.tile([128, 2 * NCOL], F32, tag=f"if{sg}")
                pg = ps.tile([128, NCOL], F32, tag=f"g{sg}")
                po = ps.tile([128, NCOL], F32, tag=f"o{sg}")
                dsts = {"i": pif[:, 0:NCOL], "f": pif[:, NCOL:2 * NCOL],
                        "g": pg[:], "o": po[:]}
                first = (s == 0)  # h0 == 0: skip the recurrence matmul
                for gi, gate in enumerate(GATES):
                    dst = dsts[gate]
                    nc.tensor.matmul(dst[:, :], wx[:, 128 * gi:128 * (gi + 1)],
                                     xs[:], start=True, stop=first,
                                     skip_group_check=True)
                    if not first:
                        nc.tensor.matmul(dst[:, :],
                                         whh[:, 128 * gi:128 * (gi + 1)],
                                         Hst[sg][:], start=False, stop=True,
                                         skip_group_check=True)

                sIF = sb.tile([128, 2 * NCOL], F32, tag=f"IF{sg}")
                sG = sb.tile([128, NCOL], F32, tag=f"G{sg}")
                sO = sb.tile([128, NCOL], F32, tag=f"O{sg}")
                nc.scalar.activation(sIF[:], pif[:], AF.Sigmoid)
                nc.scalar.activation(sG[:], pg[:], AF.Tanh)
                nc.scalar.activation(sO[:], po[:], AF.Sigmoid)

                t1 = sb.tile([128, NCOL], F32, tag=f"T1{sg}")
                t2 = sb.tile([128, NCOL], F32, tag=f"T2{sg}")
                nc.vector.tensor_mul(t1[:], sIF[:, NCOL:2 * NCOL], Cst[sg][:])
                nc.vector.tensor_mul(t2[:], sIF[:, 0:NCOL], sG[:])
                nc.vector.tensor_add(Cst[sg][:], t1[:], t2[:])
                tc_t = sb.tile([128, NCOL], F32, tag=f"TC{sg}")
                nc.scalar.activation(tc_t[:], Cst[sg][:], AF.Tanh)
                nc.vector.tensor_mul(Hst[sg][:], sO[:], tc_t[:])

            for sg in range(N_SG):
                prep_window(sg, 0)
            for w in range(N_WIN):
                if w + 1 < N_WIN:
                    for sg in range(N_SG):
                        prep_window(sg, w + 1)
                for j in range(4):
                    for sg in range(N_SG):
                        emit_step(sg, 4 * w + j)
                for sg in range(N_SG):
                    xtiles[sg][w] = None  # allow slot reuse

            # FC tail: out_T[m, col] per stream; m = 4*half + class.
            for sg in range(N_SG):
                pfc = ps.tile([8, NCOL], F32, tag=f"g{sg}")
                nc.tensor.matmul(pfc[:], wfc[:], Hst[sg][:], start=True, stop=True)
                sfc = sb.tile([8, NCOL], F32, tag=f"FC{sg}")
                nc.scalar.activation(sfc[:], pfc[:], AF.Identity, bias=fcb[:, 0:1])
                nc.sync.dma_start(out=out_d[sg], in_=sfc[:])

    nc.compile()
    return nc


def _prep_inputs(messages, embedding, W_ih, W_hh, b_ih, b_hh, fc_w, fc_b):
    """Host-side packing of weights and padded messages."""
    msgs = np.asarray(messages, dtype=np.float32)
    mp = np.zeros((B, S, VP), dtype=np.float32)
    mp[:, :, :V] = msgs
    mp[:, :, V] = 1.0  # const channel -> carries biases through xproj
    mp = mp.reshape(B, S * VP)

    # Folded input projection [VP, 4H]; row V holds the biases.
    wcomb = (np.asarray(embedding, np.float64) @ np.asarray(W_ih, np.float64).T)
    wx_full = np.zeros((VP, 4 * H), dtype=np.float32)
    wx_full[:V] = wcomb.astype(np.float32)
    wx_full[V] = (np.asarray(b_ih, np.float64)
                  + np.asarray(b_hh, np.float64)).astype(np.float32)

    # wx: [64, 4*128]: per gate a block-diag over batch halves:
    #   rows 0-31 (v of half0) -> cols 0-63, rows 32-63 (half1) -> cols 64-127.
    wx = np.zeros((64, 4 * 128), dtype=np.float32)
    for gi in range(4):
        blk = wx_full[:, 64 * gi:64 * (gi + 1)]  # [VP, 64]
        wx[0:32, 128 * gi:128 * gi + 64] = blk
        wx[32:64, 128 * gi + 64:128 * gi + 128] = blk

    # whh: [128, 4*128]: block-diag of W_hh_gate^T per gate.
    whh_np = np.asarray(W_hh, dtype=np.float32)
    whh = np.zeros((128, 4 * 128), dtype=np.float32)
    for gi in range(4):
        wg = whh_np[64 * gi:64 * (gi + 1), :]  # [64 out, 64 in]
        whh[0:64, 128 * gi:128 * gi + 64] = wg.T
        whh[64:128, 128 * gi + 64:128 * gi + 128] = wg.T

    # wfc: [128, 8]: cols 4*half + c.
    fcw = np.asarray(fc_w, dtype=np.float32)
    wfc = np.zeros((128, 8), dtype=np.float32)
    for half in range(2):
        wfc[64 * half:64 * half + 64, 4 * half:4 * half + C] = fcw.T

    fcb = np.zeros((8, 1), dtype=np.float32)
    fcb[0:C, 0] = np.asarray(fc_b, np.float32)
    fcb[4:4 + C, 0] = np.asarray(fc_b, np.float32)

    ident = np.eye(128, dtype=np.float32)

    in_maps = []
    for core in range(N_CORES):
        in_maps.append({
            "msgs": mp[core * BC:(core + 1) * BC],
            "wx": wx, "whh": whh, "wfc": wfc, "fcb": fcb, "ident": ident,
        })
    return in_maps


def _assemble(results):
    logits = np.empty((B, C), dtype=np.float32)
    for core in range(N_CORES):
        o = results[core]["out"].reshape(N_SG, 2, 4, NCOL)  # [sg, half, c4, col]
        o = np.transpose(o, (0, 1, 3, 2)).reshape(BC, 4)[:, :C]
        logits[core * BC:(core + 1) * BC] = o
    return logits


def kernel(**inputs):
    from concourse.bass_utils import run_bass_kernel_spmd

    if "nc" not in _CACHE:
        _CACHE["nc"] = _build_program()
    nc = _CACHE["nc"]
    in_maps = _prep_inputs(**inputs)
    res = run_bass_kernel_spmd(nc, in_maps, list(range(N_CORES)))
    return _assemble(res.results)
